# revision 3
# baseline (speedup 1.0000x reference)
"""Trainium2 Bass kernel for nn_Attention_31997506355363 (sparse_attention), v3.

Sharding: 8 cores = 2 batches x 4 head-groups (4 heads of 16 each).
Host sums the 4 head-group partial Wout products per batch.

v3 strategy (measured on HW: bf16 512-col mm = 221ns, fp8 = 38ns, fp8
DoubleRow = 42ns -> fp8 is ~5.8x):
  - host pre-transposes and pre-quantizes x and W to fp8e4 hi+lo residual
    pairs in a kt-interleaved layout [128, 8, N] so projections run as
    fp8 DoubleRow (two 128-row kt blocks per instruction), 4 residual
    streams (hi*hi + lo*hi + hi*lo + lo*lo) ~ bf16 precision at 3.5x speed
  - both attends' softmax shifted by c=2: e' = exp(logit - 2), sink logit
    joins as exp(sink - 2); softmax is shift-invariant. 0.125 head scale
    folded into the exp constants so k1T/k2T/qT/hT stay unscaled fp8e4
  - e tiles are fp8, produced two ways per jb block (fixed assignment):
      ACT: exact exp via activation(Exp, scale=0.125, bias=-2) -> fp8e4
      DVE: Schraudolph bit-trick, one tensor_scalar op:
           uint8(round(sim*0.125*5.7708 + 48.4632)) == e5m2 bits of
           exp(sim*0.125 - 2) (~7% RMS, fine for softmax weights)
    window safety: e5m2 path valid for logit in [-8.4, +12.9], e4m3 path
    exact-exp output in [0, 81] for logit <= 6.4 (|logit|<=6.4 observed)
  - all attend matmuls fp8 (sim1/av1/ones/sim2/av2); masks via GPSIMD
    affine_select in place on e blocks; denominators via fp8 ones-column
    matmuls as before
  - attend2's e tiles are exact bf16 (ACT), v2a bf16, v1 fp8 hi+lo residual
    (two av1 accumulation streams) -- sim-validated mix at rel err 0.0063
  - Wout / o2T stay bf16 (final output precision)
"""

import sys

for _p in ("/opt/trn_rl_repo",):
    if _p not in sys.path:
        sys.path.insert(0, _p)

import numpy as np
import concourse.bass as bass
from concourse import bacc
import concourse.mybir as mybir
from concourse.tile import TileContext

FP32 = mybir.dt.float32
BF16 = mybir.dt.bfloat16
F8 = mybir.dt.float8e4
F8E5 = mybir.dt.float8e5
U8 = mybir.dt.uint8
N_CORES = 8
N = 2048            # sequence length
DQ = 1024           # model dim
HEADS = 4           # heads per core
NB = N // 128       # 16 key blocks
PASS = 1024         # attend i-pass width (2 passes)
ACT = mybir.ActivationFunctionType
ALU = mybir.AluOpType
DR = mybir.MatmulPerfMode.DoubleRow

# exp constants: e' = exp(logit*0.125 - 2)
LOG2E = 1.4426950408889634
ACT_SCALE = 0.125
ACT_BIAS = -2.0
# e5m2 bits = 4*log2(e') + 60 = sim*(0.125*4*LOG2E) + (60 - 8*LOG2E)
E5_SCALE = 0.125 * 4.0 * LOG2E
E5_BIAS = 60.0 - 8.0 * LOG2E

# engine class per attend1 key block jb: True -> ACT exact exp (e4m3),
# False -> DVE bit-trick (e5m2).  ACT also produces ALL attend2 e tiles
# (exact exp -> bf16; attend2 needs the accuracy), so DVE takes most of
# attend1's.
JB_ACT = [jb % 4 == 3 for jb in range(NB)]   # 4 ACT, 12 DVE per 16

REPS = 1
DEBUG = False
SIM_DT = F8   # qT/k1T/k2T/hT dtype (debug knob)


def _runs_for(jb, p):
    """i-subblock runs (in 128-col units within a 1024-wide pass) that are
    not fully masked for key-block jb.  Sub-block t covers queries
    I = 8p + t; (I, jb) is fully masked iff 1 <= jb - I <= 3."""
    skip_lo = max(0, jb - 8 * p - 3)
    skip_hi = min(8, jb - 8 * p)
    if skip_lo >= skip_hi:
        return [(0, 8)], None
    runs = []
    if skip_lo > 0:
        runs.append((0, skip_lo))
    if skip_hi < 8:
        runs.append((skip_hi, 8))
    return runs, (skip_lo, skip_hi)


def build_kernel(nc, tc, io):
    mm = nc.tensor.matmul

    out = io["out"]

    const = tc.alloc_tile_pool(name="const", bufs=1)
    stat = tc.alloc_tile_pool(name="stat", bufs=1)
    xw = tc.alloc_tile_pool(name="xw", bufs=1)       # phase-1 x/W tiles
    ps_w = tc.alloc_tile_pool(name="ps_w", bufs=2, space="PSUM")   # 4 banks

    # ---- constants ----
    onescol = const.tile([128, 1], F8, tag="onescol", name="onescol")
    nc.vector.memset(onescol[:], 1.0)
    onesrow = const.tile([1, 128], FP32, tag="onesrow", name="onesrow")
    nc.vector.memset(onesrow[:], 1.0)
    ones4 = const.tile([128, HEADS], BF16, tag="ones4", name="ones4")
    nc.vector.memset(ones4[:], 1.0)
    expbias = const.tile([128, 1], FP32, tag="expbias", name="expbias")
    nc.vector.memset(expbias[:], ACT_BIAS)

    # ---- persistent SBUF intermediates (all fp8 except o2T/wout) ----
    qT_sb = [stat.tile([128, N], SIM_DT, tag=f"qT{t}", name=f"qT{t}") for t in range(2)]
    k1T_sb = [stat.tile([128, N], SIM_DT, tag=f"k1T{t}", name=f"k1T{t}") for t in range(2)]
    k2T_sb = [stat.tile([128, N], SIM_DT, tag=f"k2T{t}", name=f"k2T{t}") for t in range(4)]
    v1h_sb = [stat.tile([128, 512], F8, tag=f"v1h{t}", name=f"v1h{t}") for t in range(NB)]
    v1l_sb = [stat.tile([128, 512], F8, tag=f"v1l{t}", name=f"v1l{t}") for t in range(NB)]
    v2a_sb = [stat.tile([128, 65 * HEADS], BF16, tag=f"v2a{t}", name=f"v2a{t}")
              for t in range(NB)]
    o2T = [stat.tile([128, N], BF16, tag=f"o2T{t}", name=f"o2T{t}") for t in range(2)]

    # =====================================================================
    # Phase 1: DMA fp8 hi/lo x + W (host pre-transposed, kt-interleaved),
    # then DoubleRow residual projections.
    # =====================================================================
    # x tiles [128, 8, 2048] (chunked DMA by sequence-column range so the
    # first projection groups only wait on the columns they touch);
    # W tiles [128, 8, C] (whole)
    def xtile(nm):
        return xw.tile([128, 8, N], F8, tag=nm, name=nm)

    xq_hi, xq_lo = xtile("xq_hi"), xtile("xq_lo")
    xkv_hi, xkv_lo = xtile("xkv_hi"), xtile("xkv_lo")

    def ld_x(t, nm, c0, c1, eng):
        eng.dma_start(out=t[:, :, c0:c1], in_=io[nm][:, :, c0:c1])

    def ld_w(nm, cols, eng):
        t = xw.tile([128, 8, cols], F8, tag=nm, name=nm)
        eng.dma_start(out=t[:], in_=io[nm])
        return t

    # first half of x (cols 0:1024) + all weights, then second half
    ld_x(xq_hi, "xq_hi", 0, 512, nc.sync)
    ld_x(xq_hi, "xq_hi", 512, 1024, nc.scalar)
    ld_x(xq_lo, "xq_lo", 0, 512, nc.sync)
    ld_x(xq_lo, "xq_lo", 512, 1024, nc.scalar)
    wq_hi = ld_w("wq_hi", 256, nc.sync)
    wq_lo = ld_w("wq_lo", 256, nc.scalar)
    ld_x(xkv_hi, "xkv_hi", 0, 512, nc.sync)
    ld_x(xkv_hi, "xkv_hi", 512, 1024, nc.scalar)
    ld_x(xkv_lo, "xkv_lo", 0, 512, nc.sync)
    ld_x(xkv_lo, "xkv_lo", 512, 1024, nc.scalar)
    wk1_hi = ld_w("wk1_hi", 256, nc.sync)
    wk1_lo = ld_w("wk1_lo", 256, nc.scalar)
    wk2_hi = ld_w("wk2_hi", 512, nc.sync)
    wk2_lo = ld_w("wk2_lo", 512, nc.scalar)
    wv1_hi = ld_w("wv1_hi", 512, nc.sync)
    wv1_lo = ld_w("wv1_lo", 512, nc.scalar)
    wv2_hi = ld_w("wv2_hi", 256, nc.sync)
    wv2_lo = ld_w("wv2_lo", 256, nc.scalar)
    wv2_3 = ld_w("wv2_3", 256, nc.sync)
    xkv_3 = xtile("xkv_3")
    for c in range(4):
        ld_x(xkv_3, "xkv_3", c * 512, (c + 1) * 512,
             nc.sync if c % 2 == 0 else nc.scalar)
    for t, nm, eng in ((xq_hi, "xq_hi", nc.sync), (xq_lo, "xq_lo", nc.scalar),
                       (xkv_hi, "xkv_hi", nc.sync), (xkv_lo, "xkv_lo", nc.scalar)):
        ld_x(t, nm, 1024, 1536, eng)
        ld_x(t, nm, 1536, 2048, eng)
    wout_sb = [stat.tile([128, DQ], BF16, tag=f"wo{t}", name=f"wo{t}")
               for t in range(2)]
    for t in range(2):
        nc.scalar.dma_start(out=wout_sb[t][:],
                            in_=io["wout"][t * 128:(t + 1) * 128, :])
    sink_sb = const.tile([1, HEADS], FP32, tag="sink", name="sink")
    nc.scalar.dma_start(out=sink_sb[:], in_=io["sink"][:])
    esink = const.tile([1, HEADS], FP32, tag="esink", name="esink")
    nc.scalar.activation(esink[:], sink_sb[:], ACT.Exp, bias=expbias[0:1, :],
                         scale=1.0)

    # stationary-W groups: q (2 m-slices), k1 (2), k2 (4)
    def proj_groups(hf):
        groups = (
            [(qT_sb[m], wq_hi, wq_lo, m) for m in range(2)]
            + [(k1T_sb[m], wk1_hi, wk1_lo, m) for m in range(2)]
            + [(k2T_sb[m], wk2_hi, wk2_lo, m) for m in range(4)]
        )
        for gi, (dst, w_hi, w_lo, m) in enumerate(groups):
            acc = ps_w.tile([128, PASS], FP32, tag="pw", name="pw")
            for cb in range(2):
                c0 = hf * 1024 + cb * 512
                xh = xq_hi if gi < 2 else xkv_hi
                xl = xq_lo if gi < 2 else xkv_lo
                streams = ((w_hi, xh), (w_lo, xh), (w_hi, xl), (w_lo, xl))
                for si, (ws, xs) in enumerate(streams):
                    for j in range(4):
                        mm(acc[:, cb * 512:(cb + 1) * 512],
                           ws[:, 2 * j:2 * j + 2, m * 128:(m + 1) * 128],
                           xs[:, 2 * j:2 * j + 2, c0:c0 + 512],
                           start=(si == 0 and j == 0),
                           stop=(si == 3 and j == 3), perf_mode=DR)
            cols = slice(hf * 1024, (hf + 1) * 1024)
            if gi % 2 == 0:
                nc.vector.tensor_copy(dst[:, cols], acc[:])
            else:
                nc.scalar.copy(dst[:, cols], acc[:])

    # stationary-x groups: v1 + v2 fused per seq block nb
    def proj_v(hf):
        for nb in range(8 * hf, 8 * hf + 8):
            acc = ps_w.tile([128, PASS], FP32, tag="pw", name="pw")
            streams = ((xkv_hi, wv1_hi, wv2_hi), (xkv_lo, wv1_hi, wv2_hi),
                       (xkv_hi, wv1_lo, wv2_lo), (xkv_lo, wv1_lo, wv2_lo))
            for si, (xs, w1, w2) in enumerate(streams):
                for j in range(4):
                    mm(acc[:, 0:512],
                       xs[:, 2 * j:2 * j + 2, nb * 128:(nb + 1) * 128],
                       w1[:, 2 * j:2 * j + 2, :],
                       start=(si == 0 and j == 0), stop=(si == 3 and j == 3),
                       perf_mode=DR)
                    mm(acc[:, 512:768],
                       xs[:, 2 * j:2 * j + 2, nb * 128:(nb + 1) * 128],
                       w2[:, 2 * j:2 * j + 2, :],
                       start=(si == 0 and j == 0), stop=False,
                       perf_mode=DR)
            # v2 output feeds the final output directly: two third-level
            # residual streams (x3*W_hi + x_hi*W3) push it to ~bf16 precision
            for si, (xs, w2) in enumerate(((xkv_3, wv2_hi), (xkv_hi, wv2_3))):
                for j in range(4):
                    mm(acc[:, 512:768],
                       xs[:, 2 * j:2 * j + 2, nb * 128:(nb + 1) * 128],
                       w2[:, 2 * j:2 * j + 2, :],
                       start=False, stop=(si == 1 and j == 3),
                       perf_mode=DR)
            if nb % 2 == 0:
                nc.vector.tensor_copy(v1h_sb[nb][:], acc[:, 0:512])
            else:
                nc.scalar.copy(v1h_sb[nb][:], acc[:, 0:512])
            nc.vector.tensor_sub(v1l_sb[nb][:], acc[:, 0:512], v1h_sb[nb][:])
            sv = v2a_sb[nb][:].rearrange("p (h c) -> p h c", h=HEADS)
            nc.vector.tensor_copy(
                sv[:, :, 0:64],
                acc[:, 512:768].rearrange("p (h c) -> p h c", h=HEADS))
            nc.vector.tensor_copy(
                sv[:, :, 64:65],
                ones4[:].rearrange("p (h c) -> p h c", h=HEADS))

    proj_groups(0)
    proj_v(0)
    proj_groups(1)
    proj_v(1)

    ps_w.release()
    xw.release()

    # attend-phase pools
    e1p = tc.alloc_tile_pool(name="e1", bufs=1)    # 16 resident e tiles
    epool = tc.alloc_tile_pool(name="e", bufs=3)
    npool = tc.alloc_tile_pool(name="nrm", bufs=2)
    osb_p = tc.alloc_tile_pool(name="osb", bufs=2)
    ps_a = tc.alloc_tile_pool(name="ps_a", bufs=2, space="PSUM")   # 4 banks
    ps_b = tc.alloc_tile_pool(name="ps_b", bufs=1, space="PSUM")   # 2 banks
    ps_on = tc.alloc_tile_pool(name="ps_on", bufs=1, space="PSUM")  # 1 bank
    ps_bc = tc.alloc_tile_pool(name="ps_bc", bufs=1, space="PSUM")  # 1 bank
    _pools2 = [e1p, epool, npool, osb_p, ps_a, ps_b, ps_on, ps_bc]

    # =====================================================================
    # Phase 2: attends, all-fp8 matmuls
    # =====================================================================
    def emit_e(e, simp, jb, p, bf=False):
        """Write e' = exp(logit - 2) (logit = sim*0.125) into tile e, masked.
        bf=True (attend2): ACT exact exp -> bf16.  Else fp8 per JB class:
        ACT exact -> e4m3, or DVE Schraudolph bit-trick -> e5m2 bits."""
        runs, skip = _runs_for(jb, p)
        if bf:
            for (t0, t1) in runs:
                nc.scalar.activation(e[:, t0 * 128:t1 * 128],
                                     simp[:, t0 * 128:t1 * 128], ACT.Exp,
                                     bias=expbias[:], scale=ACT_SCALE)
        elif JB_ACT[jb]:
            for (t0, t1) in runs:
                nc.scalar.activation(e[:, t0 * 128:t1 * 128].bitcast(F8),
                                     simp[:, t0 * 128:t1 * 128], ACT.Exp,
                                     bias=expbias[:], scale=ACT_SCALE)
        else:
            for (t0, t1) in runs:
                nc.vector.tensor_scalar(
                    out=e[:, t0 * 128:t1 * 128], in0=simp[:, t0 * 128:t1 * 128],
                    scalar1=E5_SCALE, scalar2=E5_BIAS,
                    op0=ALU.mult, op1=ALU.add)
        if skip is not None:
            nc.gpsimd.memset(e[:, skip[0] * 128:skip[1] * 128], 0)
        cls_dt = BF16 if bf else (F8 if JB_ACT[jb] else F8E5)
        td = jb - 8 * p
        if 0 <= td < 8:   # diagonal block: keep jj <= ii
            blk = e[:, td * 128:(td + 1) * 128]
            blk = blk if bf else blk.bitcast(cls_dt)
            nc.gpsimd.affine_select(
                out=blk, in_=blk, compare_op=ALU.is_ge,
                fill=0.0, base=0, pattern=[[1, 128]], channel_multiplier=-1)
        ta = jb - 4 - 8 * p
        if 0 <= ta < 8:   # jb == I+4 block: keep jj > ii
            blk = e[:, ta * 128:(ta + 1) * 128]
            blk = blk if bf else blk.bitcast(cls_dt)
            nc.gpsimd.affine_select(
                out=blk, in_=blk, compare_op=ALU.is_ge,
                fill=0.0, base=-1, pattern=[[-1, 128]], channel_multiplier=1)
        return e

    def e_mm_ap(e, jb, sl):
        return e[:, sl].bitcast(F8 if JB_ACT[jb] else F8E5)

    def sim_exp_1(h, p):
        """Attend1 S-stage: sims -> e' into 16 resident fp8 tiles."""
        hh = 64 * (h % 2)
        k1h = k1T_sb[h // 2][hh:hh + 64, :]
        qh = qT_sb[h // 2][hh:hh + 64, p * PASS:(p + 1) * PASS]
        es = []
        for jb in range(NB):
            simp = ps_a.tile([128, PASS], FP32, tag="sim", name="sim")
            for col in (0, 512):
                mm(simp[:, col:col + 512],
                   k1h[:, jb * 128:(jb + 1) * 128],
                   qh[:, col:col + 512],
                   start=True, stop=True)
            e = e1p.tile([128, PASS], U8, tag=f"e1_{jb}", name=f"e1_{jb}")
            emit_e(e, simp, jb, p)
            es.append(e)
        return es

    def attend2(h, p, hT, out2):
        """Fused jb-pipelined attend2: sim -> e' -> v2a.T @ e."""
        k2h = k2T_sb[h][:]

        def do_sim(jb):
            simp = ps_a.tile([128, PASS], FP32, tag="sim", name="sim")
            for col in (0, 512):
                mm(simp[:, col:col + 512],
                   k2h[:, jb * 128:(jb + 1) * 128],
                   hT[:, col:col + 512],
                   start=True, stop=True)
            return simp

        def do_av(jb, e):
            for s in range(2):
                mm(out2[:, s * 512:(s + 1) * 512],
                   v2a_sb[jb][:, 65 * h:65 * h + 65],
                   e[:, s * 512:(s + 1) * 512],
                   start=(jb == 0), stop=(jb == NB - 1))

        prev = None
        for jb in range(NB):
            simp = do_sim(jb)
            if prev is not None:
                do_av(jb - 1, prev[0])
            e = epool.tile([128, PASS], BF16, tag="e", name="e")
            emit_e(e, simp, jb, p, bf=True)
            prev = (e, jb)
        do_av(NB - 1, prev[0])

    def wout_half(p):
        for nb in range(8 * p, 8 * p + 8):
            pool, tag = (ps_b, "av") if nb % 2 == 0 else (ps_a, "sim")
            acc = pool.tile([128, PASS], FP32, tag=tag, name=tag)
            for s in range(2):
                for kt in range(2):
                    mm(acc[:, s * 512:(s + 1) * 512],
                       o2T[kt][:, nb * 128:(nb + 1) * 128],
                       wout_sb[kt][:, s * 512:(s + 1) * 512],
                       start=(kt == 0), stop=(kt == 1))
            osb = osb_p.tile([128, DQ], FP32, tag="osb", name="osb")
            if nb % 2 == 0:
                nc.vector.tensor_copy(osb[:], acc[:])
            else:
                nc.scalar.copy(osb[:], acc[:])
            nc.sync.dma_start(out=out[nb * 128:(nb + 1) * 128, :], in_=osb[:])

    passes = [(h, p) for p in range(2) for h in range(HEADS)]
    e1s = sim_exp_1(*passes[0])
    e1s_dbg = e1s
    for idx, (h, p) in enumerate(passes):
        hh = 64 * (h % 2)

        # ------------- attend 1 V-stage: av + denominator matmuls ---------
        out1 = ps_b.tile([128, PASS], FP32, tag="av", name="av")
        ones = ps_on.tile([33, 512], FP32, tag="ones", name="ones")
        for jb in range(NB):
            for s in range(2):
                for vt in (v1h_sb, v1l_sb):
                    mm(out1[:, s * 512:(s + 1) * 512],
                       vt[jb][:, 128 * h:128 * h + 128],
                       e_mm_ap(e1s[jb], jb, slice(s * 512, (s + 1) * 512)),
                       start=(jb == 0 and vt is v1h_sb), stop=(jb == NB - 1 and vt is v1l_sb))
            for s in range(2):
                mm(ones[32 * s:32 * s + 1, :], onescol[:],
                   e_mm_ap(e1s[jb], jb, slice(s * 512, (s + 1) * 512)),
                   start=(jb == 0), stop=(jb == NB - 1))

        # normalize (z = out1 / denom) + silu -> hT (fp8), per 512-col half
        zf = npool.tile([128, PASS], FP32, tag="z", name="z")
        rbs = npool.tile([128, PASS], FP32, tag="rb", name="rb")
        tql = npool.tile([128, PASS], FP32, tag="tq", name="tq")
        hT = npool.tile([128, PASS], SIM_DT, tag="hT", name="hT")
        for s_ in range(2):
            sl = slice(s_ * 512, (s_ + 1) * 512)
            ds_ = npool.tile([1, PASS], FP32, tag="ds", name="ds")
            nc.vector.tensor_copy(ds_[0:1, 0:512], ones[32 * s_:32 * s_ + 1, :])
            nc.vector.tensor_scalar_add(ds_[0:1, 0:512], ds_[0:1, 0:512],
                                        esink[0:1, h:h + 1])
            nc.vector.reciprocal_approx_fast(ds_[0:1, 0:512], ds_[0:1, 0:512])
            rbp = ps_bc.tile([128, 512], FP32, tag="bc", name="bc")
            mm(rbp[:], onesrow[:], ds_[0:1, 0:512], start=True, stop=True)
            nc.scalar.copy(rbs[:, sl], rbp[:])
            nc.vector.tensor_mul(zf[:, sl], out1[:, sl], rbs[:, sl])
            nc.scalar.activation(tql[:, sl], zf[:, sl], ACT.Exp, scale=-1.0)
            nc.vector.tensor_scalar_add(tql[:, sl], tql[:, sl], 1.0)
            nc.vector.reciprocal_approx_fast(tql[:, sl], tql[:, sl])
            nc.vector.tensor_mul(hT[:, sl], zf[:, sl], tql[:, sl])

        # next pass's S-stage: fills the PE while the chain above runs
        if idx + 1 < len(passes):
            e1s = sim_exp_1(*passes[idx + 1])

        # ------------- attend 2 (fused jb-pipelined) -------------
        out2 = ps_b.tile([65, PASS], FP32, tag="av", name="av")
        attend2(h, p, hT[:], out2[:])

        # normalize attend2 (denominator rode along as row 64)
        d2 = npool.tile([1, PASS], FP32, tag="ds", name="ds")
        nc.vector.tensor_copy(d2[:], out2[64:65, :])
        nc.vector.tensor_scalar_add(d2[:], d2[:], esink[0:1, h:h + 1])
        nc.vector.reciprocal_approx_fast(d2[:], d2[:])
        rbs2 = npool.tile([64, PASS], FP32, tag="rb2", name="rb2")
        for s_ in range(2):
            rbp = ps_bc.tile([128, 512], FP32, tag="bc", name="bc")
            mm(rbp[0:64, :], onesrow[0:1, 0:64],
               d2[0:1, s_ * 512:(s_ + 1) * 512], start=True, stop=True)
            nc.scalar.copy(rbs2[:, s_ * 512:(s_ + 1) * 512], rbp[0:64, :])
        dst = o2T[h // 2][hh:hh + 64, p * PASS:(p + 1) * PASS]
        nc.vector.tensor_mul(dst, out2[0:64, :], rbs2[:])

        if DEBUG and h == 0 and p == 0:
            dbg = npool.tile([128, PASS], FP32, tag="dbg", name="dbg")
            nc.vector.tensor_copy(dbg[:], out1[:])
            nc.sync.dma_start(out=io["dbg_out1"], in_=dbg[:])
            dbgo = npool.tile([65, PASS], FP32, tag="dbgo", name="dbgo")
            nc.vector.tensor_copy(dbgo[:], out2[:])
            nc.sync.dma_start(out=io["dbg_out2"], in_=dbgo[:])
            dbgn = npool.tile([33, 512], FP32, tag="dbgn", name="dbgn")
            nc.vector.tensor_copy(dbgn[:], ones[:])
            nc.sync.dma_start(out=io["dbg_ones"], in_=dbgn[:])
            nc.sync.dma_start(out=io["dbg_hT"], in_=hT[:].bitcast(U8))
            nc.sync.dma_start(out=io["dbg_e10"], in_=e1s_dbg[0][:])
            nc.sync.dma_start(out=io["dbg_e15"], in_=e1s_dbg[5][:])

        # interleave the output projection for the completed column half
        if idx == len(passes) - 1 or (idx + 1 < len(passes)
                                      and passes[idx + 1][1] != p):
            wout_half(p)

    for p_ in reversed(_pools2):
        p_.release()
    for p_ in (stat, const):
        p_.release()


_NC_CACHE = {}


def build_nc():
    key = (REPS, DEBUG, str(SIM_DT))
    if key in _NC_CACHE:
        return _NC_CACHE[key]
    nc = bacc.Bacc("TRN2", target_bir_lowering=False, debug=False,
                   num_devices=N_CORES)
    io = {}
    for nm in ("xq_hi", "xq_lo", "xkv_hi", "xkv_lo", "xkv_3"):
        io[nm] = nc.dram_tensor(nm, [128, 8, N], F8, kind="ExternalInput").ap()
    io["wv2_3"] = nc.dram_tensor("wv2_3", [128, 8, 256], F8,
                                 kind="ExternalInput").ap()
    for nm, c in (("wq", 256), ("wk1", 256), ("wv1", 512), ("wk2", 512),
                  ("wv2", 256)):
        io[nm + "_hi"] = nc.dram_tensor(nm + "_hi", [128, 8, c], F8,
                                        kind="ExternalInput").ap()
        io[nm + "_lo"] = nc.dram_tensor(nm + "_lo", [128, 8, c], F8,
                                        kind="ExternalInput").ap()
    io["wout"] = nc.dram_tensor("wout", [256, DQ], BF16, kind="ExternalInput").ap()
    io["sink"] = nc.dram_tensor("sink", [1, HEADS], FP32, kind="ExternalInput").ap()
    io["out"] = nc.dram_tensor("out", [N, DQ], FP32, kind="ExternalOutput").ap()
    if DEBUG:
        io["dbg_out1"] = nc.dram_tensor("dbg_out1", [128, PASS], FP32, kind="ExternalOutput").ap()
        io["dbg_out2"] = nc.dram_tensor("dbg_out2", [65, PASS], FP32, kind="ExternalOutput").ap()
        io["dbg_ones"] = nc.dram_tensor("dbg_ones", [33, 512], FP32, kind="ExternalOutput").ap()
        io["dbg_hT"] = nc.dram_tensor("dbg_hT", [128, PASS], mybir.dt.uint8, kind="ExternalOutput").ap()
        io["dbg_e10"] = nc.dram_tensor("dbg_e10", [128, PASS], mybir.dt.uint8, kind="ExternalOutput").ap()
        io["dbg_e15"] = nc.dram_tensor("dbg_e15", [128, PASS], mybir.dt.uint8, kind="ExternalOutput").ap()
    if REPS == 0:
        io["dummy0"] = nc.dram_tensor("dummy0", [1, 8], FP32,
                                      kind="ExternalInput").ap()
    with TileContext(nc) as tc:
        if REPS == 0:
            pool0 = tc.alloc_tile_pool(name="p0", bufs=1)
            t0_ = pool0.tile([128, N], F8, name="t0_")
            nc.sync.dma_start(out=t0_[:], in_=io["xq_hi"][:, 0, :])
            o0_ = pool0.tile([128, DQ], FP32, name="o0_")
            nc.vector.tensor_copy(o0_[:], t0_[:, 0:DQ])
            for nb in range(NB):
                nc.sync.dma_start(out=io["out"][nb * 128:(nb + 1) * 128, :],
                                  in_=o0_[:])
            pool0.release()
        for _ in range(REPS):
            build_kernel(nc, tc, io)
    nc.compile()
    _NC_CACHE[key] = (nc, io)
    return nc, io


_DT = None


def _dts():
    global _DT
    if _DT is None:
        import ml_dtypes
        _DT = (np.dtype(ml_dtypes.bfloat16), np.dtype(ml_dtypes.float8_e4m3))
    return _DT


def _hi_lo(a, f8):
    hi = a.astype(f8)
    lo = (a - hi.astype(np.float32)).astype(f8)
    return np.ascontiguousarray(hi), np.ascontiguousarray(lo)


def _third(a, hi, lo, f8):
    return np.ascontiguousarray(
        (a - hi.astype(np.float32) - lo.astype(np.float32)).astype(f8))


def make_in_maps(inputs):
    bf, f8 = _dts()
    # x_re[p, kt, n] = x[n, kt*128+p]
    def x_re(x):
        return _hi_lo(np.ascontiguousarray(
            x.T.reshape(8, 128, N).transpose(1, 0, 2)), f8)
    # w_re[p, kt, c] = W[kt*128+p, c]
    def w_re(w):
        return _hi_lo(np.ascontiguousarray(
            w.reshape(8, 128, -1).transpose(1, 0, 2)), f8)

    xq_b = [x_re(np.asarray(inputs["queries_input"][b], np.float32))
            for b in range(2)]
    xkv_arr = [np.ascontiguousarray(
        np.asarray(inputs["key_values_input"][b], np.float32)
        .T.reshape(8, 128, N).transpose(1, 0, 2)) for b in range(2)]
    xkv_b = [_hi_lo(a, f8) for a in xkv_arr]
    xkv_3b = [_third(a, h, l, f8) for a, (h, l) in zip(xkv_arr, xkv_b)]
    in_maps = []
    for c in range(N_CORES):
        b, g = c // 4, c % 4
        s64 = slice(g * 256, (g + 1) * 256)
        s128 = slice(g * 512, (g + 1) * 512)
        wq_h, wq_l = w_re(inputs["Wq"][:, s64])
        wk1_h, wk1_l = w_re(inputs["Wk1"][:, s64])
        wv1_h, wv1_l = w_re(inputs["Wv1"][:, s128])
        wk2_h, wk2_l = w_re(inputs["Wk2"][:, s128])
        wv2_arr = np.ascontiguousarray(
            np.asarray(inputs["Wv2"][:, s64], np.float32)
            .reshape(8, 128, -1).transpose(1, 0, 2))
        wv2_h, wv2_l = _hi_lo(wv2_arr, f8)
        wv2_3 = _third(wv2_arr, wv2_h, wv2_l, f8)
        in_maps.append({
            "xq_hi": xq_b[b][0], "xq_lo": xq_b[b][1],
            "xkv_hi": xkv_b[b][0], "xkv_lo": xkv_b[b][1],
            "xkv_3": xkv_3b[b], "wv2_3": wv2_3,
            "wq_hi": wq_h, "wq_lo": wq_l,
            "wk1_hi": wk1_h, "wk1_lo": wk1_l,
            "wv1_hi": wv1_h, "wv1_lo": wv1_l,
            "wk2_hi": wk2_h, "wk2_lo": wk2_l,
            "wv2_hi": wv2_h, "wv2_lo": wv2_l,
            "wout": np.ascontiguousarray(inputs["Wout"][s64, :]).astype(bf),
            "sink": np.ascontiguousarray(
                inputs["attn_sink"][g * 4:(g + 1) * 4]).reshape(1, HEADS)
                .astype(np.float32),
        })
    return in_maps


def kernel(**inputs):
    from concourse.bass_utils import run_bass_kernel_spmd

    inputs = {k: np.asarray(v) for k, v in inputs.items()}
    nc, _ = build_nc()
    in_maps = make_in_maps(inputs)
    res = run_bass_kernel_spmd(nc, in_maps, list(range(N_CORES)))
    out = np.zeros((2, N, DQ), dtype=np.float32)
    for c in range(N_CORES):
        out[c // 4] += res.results[c]["out"]
    return out


# revision 5
# speedup vs baseline: 1.4718x; 1.4718x over previous
"""Trainium2 Bass kernel for nn_Attention_31997506355363 (sparse_attention).

Sharding: 8 cores = 2 batches x 4 head-groups (4 heads of 16 each).
Each core computes its batch's full-sequence double-attend for its 4 heads,
plus the partial output projection (Wout rows for its heads); host sums the
4 head-group partials per batch.

Math notes (verified vs reference):
  - mask keeps j<=i OR j>i+512  (the strip i<j<=i+512 is masked out)
  - softmax has a per-head sink logit in the denominator only
  - |sim| <= ~6.4 so softmax runs without max-subtraction: p = exp(sim),
    denom = sum_j p + exp(sink)
  - attends are computed transposed: simT[j,i] tiles -> exp -> outT
    accumulated as v.T @ p per 128-j-block (contraction always on the
    partition dim, so no attention-matrix transposes are needed, and
    attend1's output hiddensT feeds attend2 directly)

Perf structure (v2):
  - all matmul operands bf16 (fp32 PE runs at 1/4 rate; tolerance is 2e-2)
  - x transposed by XBAR DMA-transpose (2-byte dtype) straight into SBUF;
    no PE transposes, no PSUM->SBUF copies for xT
  - everything SBUF-resident between phases; weights loaded once;
    phase-1-only pools (xT, projection weights, wide PSUM accs) released
    before the attends
  - projections run stationary-major (one Ldweights per (w-slice), 4
    full-width moving matmuls) to cut PE sequencer pressure
  - masking via DVE multiplies with constant 0/1 triangular tiles + DVE
    memsets; GPSIMD only does one-time constant setup
  - softmax denominators: ones-row matmuls accumulate alongside v.T @ e;
    reciprocal broadcast back to 128 partitions via a rank-1 PE matmul
"""

import sys

for _p in ("/opt/trn_rl_repo",):
    if _p not in sys.path:
        sys.path.insert(0, _p)

import numpy as np
import concourse.bass as bass
from concourse import bacc
import concourse.mybir as mybir
from concourse.tile import TileContext
from concourse.masks import make_identity

FP32 = mybir.dt.float32
MM_DT = mybir.dt.bfloat16
N_CORES = 8
N = 2048            # sequence length
DQ = 1024           # model dim
HEADS = 4           # heads per core
SCALE = 0.125       # 64 ** -0.5, folded into k1T / k2T at projection copy
NB = N // 128       # 16 key blocks
PASS = 1024         # attend i-pass width (2 passes)
ACT = mybir.ActivationFunctionType

DEBUG = False
REPS = 1
PROJ_ONLY = False   # timing experiment: stop after projections


def _runs_for(jb, p):
    """i-subblock runs (in 128-col units within a 1024-wide pass) that are
    not fully masked for key-block jb.  Sub-block t covers queries
    I = 8p + t; (I, jb) is fully masked iff 1 <= jb - I <= 3."""
    skip_lo = max(0, jb - 8 * p - 3)
    skip_hi = min(8, jb - 8 * p)
    if skip_lo >= skip_hi:
        return [(0, 8)], None
    runs = []
    if skip_lo > 0:
        runs.append((0, skip_lo))
    if skip_hi < 8:
        runs.append((skip_hi, 8))
    return runs, (skip_lo, skip_hi)


def _mm_runs(jb, p):
    """Non-masked col ranges (elements, within the 1024-wide pass) for
    key-block jb, split at the 512 psum-bank boundary."""
    runs, _ = _runs_for(jb, p)
    out = []
    for (t0, t1) in runs:
        c0, c1 = t0 * 128, t1 * 128
        for h0, h1 in ((0, 512), (512, 1024)):
            a, b = max(c0, h0), min(c1, h1)
            if a < b:
                out.append((a, b))
    return out


def build_kernel(nc, tc, io):
    mm = nc.tensor.matmul

    xq, xkv = io["xq"], io["xkv"]
    wq, wk1, wv1, wk2, wv2, wout, sink = (
        io["wq"], io["wk1"], io["wv1"], io["wk2"], io["wv2"], io["wout"],
        io["sink"],
    )
    out = io["out"]

    const = tc.alloc_tile_pool(name="const", bufs=1)
    stat = tc.alloc_tile_pool(name="stat", bufs=1)
    # phase-1-only pools (released before the attends)
    xt_p = tc.alloc_tile_pool(name="xt", bufs=1)
    xin = tc.alloc_tile_pool(name="xin", bufs=1)
    wpool = tc.alloc_tile_pool(name="w", bufs=1)
    ps_w = tc.alloc_tile_pool(name="ps_w", bufs=2, space="PSUM")   # 4 banks
    ps_tp = tc.alloc_tile_pool(name="ps_tp", bufs=2, space="PSUM")  # 2 banks

    ident = const.tile([128, 128], MM_DT, tag="ident", name="ident")
    make_identity(nc, ident[:])

    # ---- constants ----
    onescol = const.tile([128, 1], MM_DT, tag="onescol", name="onescol")
    nc.vector.memset(onescol[:], 1.0)
    onesrow = const.tile([1, 128], FP32, tag="onesrow", name="onesrow")
    nc.vector.memset(onesrow[:], 1.0)
    ones4 = const.tile([128, HEADS], MM_DT, tag="ones4", name="ones4")
    nc.vector.memset(ones4[:], 1.0)

    # 0/1 triangular masks (e layout is [j partitions, i cols]):
    # tri_le keeps jj <= ii (diagonal block), tri_gt keeps jj > ii (block I+4)
    tri_le = const.tile([128, 128], MM_DT, tag="tri_le", name="tri_le")
    nc.gpsimd.memset(tri_le[:], 1.0)
    nc.gpsimd.affine_select(
        out=tri_le[:], in_=tri_le[:], compare_op=mybir.AluOpType.is_ge,
        fill=0.0, base=0, pattern=[[1, 128]], channel_multiplier=-1)
    tri_gt = const.tile([128, 128], MM_DT, tag="tri_gt", name="tri_gt")
    nc.gpsimd.memset(tri_gt[:], 1.0)
    nc.gpsimd.affine_select(
        out=tri_gt[:], in_=tri_gt[:], compare_op=mybir.AluOpType.is_ge,
        fill=0.0, base=-1, pattern=[[-1, 128]], channel_multiplier=1)

    # ---- weights (DMAs ordered around the transposes; see below) ----
    def load_w(w_dram, cols, nm, eng):
        wt = [wpool.tile([128, cols], MM_DT, tag=f"{nm}{kt}", name=f"{nm}{kt}")
              for kt in range(8)]
        for kt in range(8):
            e = eng if not isinstance(eng, tuple) else eng[kt % 2]
            e.dma_start(out=wt[kt][:], in_=w_dram[kt * 128:(kt + 1) * 128, :])
        return wt

    wq_sb = load_w(wq, 256, "wq", (nc.sync, nc.scalar))

    # ---- persistent SBUF intermediates ----
    qT_sb = [stat.tile([128, N], MM_DT, tag=f"qT{t}", name=f"qT{t}") for t in range(2)]
    k1T_sb = [stat.tile([128, N], MM_DT, tag=f"k1T{t}", name=f"k1T{t}") for t in range(2)]
    k2T_sb = [stat.tile([128, N], MM_DT, tag=f"k2T{t}", name=f"k2T{t}") for t in range(4)]
    v1_sb = [stat.tile([128, 512], MM_DT, tag=f"v1_{t}", name=f"v1_{t}") for t in range(NB)]
    v2a_sb = [stat.tile([128, 65 * HEADS], MM_DT, tag=f"v2a{t}", name=f"v2a{t}")
              for t in range(NB)]
    o2T = [stat.tile([128, N], MM_DT, tag=f"o2T{t}", name=f"o2T{t}") for t in range(2)]

    # =====================================================================
    # Phase 1: DMA-transpose x into SBUF, then stationary-major projections.
    # =====================================================================
    xqT = [xt_p.tile([128, N], MM_DT, tag=f"xqT{kt}", name=f"xqT{kt}")
           for kt in range(8)]
    xkvT = [xt_p.tile([128, N], MM_DT, tag=f"xkvT{kt}", name=f"xkvT{kt}")
            for kt in range(8)]

    def load_chunk(x_dram, c, qi):
        nat = []
        for nbl in range(4):
            r0 = c * 512 + nbl * 128
            t = xin.tile([128, DQ], MM_DT, tag=f"x{qi}{nbl}", name=f"x{qi}{nbl}")
            eng = nc.sync if (nbl % 2 == 0) else nc.scalar
            eng.dma_start(out=t[:], in_=x_dram[r0:r0 + 128, :])
            nat.append(t)
        return nat

    def transpose_nat(nat, xT, c):
        """PE-transpose a loaded 512-row chunk into xT[kt][:, c-cols].
        (The XBAR DMA-transpose path raced with compute consumers on HW —
        its completion semaphore does not reliably gate reads.)"""
        for kt in range(8):
            ps = ps_tp.tile([128, 512], MM_DT, tag="tp", name="tp")
            for nbl in range(4):
                nc.tensor.transpose(
                    ps[:, nbl * 128:(nbl + 1) * 128],
                    nat[nbl][:, kt * 128:(kt + 1) * 128], ident[:])
            if kt % 2 == 0:
                nc.vector.tensor_copy(xT[kt][:, c * 512:(c + 1) * 512], ps[:])
            else:
                nc.scalar.copy(xT[kt][:, c * 512:(c + 1) * 512], ps[:])

    def load_rest_of_weights():
        # emitted after the first chunk's x loads so the PE isn't starved
        # at startup waiting for transposable data behind 40 weight DMAs
        w = {}
        w["k1"] = load_w(wk1, 256, "wk1", nc.sync)
        w["k2"] = load_w(wk2, 512, "wk2", nc.scalar)
        w["v1"] = load_w(wv1, 512, "wv1", nc.sync)
        w["v2"] = load_w(wv2, 256, "wv2", nc.scalar)
        w["out"] = [stat.tile([128, DQ], MM_DT, tag=f"wo{t}", name=f"wo{t}")
                    for t in range(2)]
        for t in range(2):
            nc.scalar.dma_start(out=w["out"][t][:],
                                in_=wout[t * 128:(t + 1) * 128, :])
        sink_sb = const.tile([1, HEADS], FP32, tag="sink", name="sink")
        nc.scalar.dma_start(out=sink_sb[:], in_=sink[:])
        esink = const.tile([1, HEADS], FP32, tag="esink", name="esink")
        nc.scalar.activation(esink[:], sink_sb[:], ACT.Exp)
        return w, esink

    # q/k1/k2 groups: stationary-major (one Ldweights per (w-slice, kt, half),
    # two 512-wide moving matmuls); v1+v2 fused on a shared stationary.
    def proj_groups(hf):
        groups = (
            [(qT_sb[m], wq_sb, m, xqT, None) for m in range(2)]
            + [(k1T_sb[m], wk1_sb, m, xkvT, SCALE) for m in range(2)]
            + [(k2T_sb[m], wk2_sb, m, xkvT, SCALE) for m in range(4)]
        )
        cols = slice(hf * 1024, (hf + 1) * 1024)
        for gi, (dst, wsb, m, xT, scale) in enumerate(groups):
            acc = ps_w.tile([128, PASS], FP32, tag="pw", name="pw")
            for kt in range(8):
                for cb in range(2):
                    c0 = hf * 1024 + cb * 512
                    mm(acc[:, cb * 512:(cb + 1) * 512],
                       wsb[kt][:, m * 128:(m + 1) * 128],
                       xT[kt][:, c0:c0 + 512],
                       start=(kt == 0), stop=(kt == 7))
            if scale is None:
                if gi % 2 == 0:
                    nc.vector.tensor_copy(dst[:, cols], acc[:])
                else:
                    nc.scalar.copy(dst[:, cols], acc[:])
            else:
                if gi % 2 == 0:
                    nc.vector.tensor_scalar_mul(dst[:, cols], acc[:], scale)
                else:
                    nc.scalar.mul(dst[:, cols], acc[:], scale)

    def proj_v(hf):
        for nb in range(8 * hf, 8 * hf + 8):
            acc = ps_w.tile([128, PASS], FP32, tag="pw", name="pw")
            for kt in range(8):
                mm(acc[:, 0:512], xkvT[kt][:, nb * 128:(nb + 1) * 128], wv1_sb[kt][:],
                   start=(kt == 0), stop=(kt == 7))
                mm(acc[:, 512:768], xkvT[kt][:, nb * 128:(nb + 1) * 128], wv2_sb[kt][:],
                   start=(kt == 0), stop=(kt == 7))
            if nb % 2 == 0:
                nc.vector.tensor_copy(v1_sb[nb][:], acc[:, 0:512])
            else:
                nc.scalar.copy(v1_sb[nb][:], acc[:, 0:512])
            # pack v2 [h*64 cols] into 65-col groups with a ones column
            sv = v2a_sb[nb][:].rearrange("p (h c) -> p h c", h=HEADS)
            nc.vector.tensor_copy(
                sv[:, :, 0:64],
                acc[:, 512:768].rearrange("p (h c) -> p h c", h=HEADS))
            nc.vector.tensor_copy(
                sv[:, :, 64:65],
                ones4[:].rearrange("p (h c) -> p h c", h=HEADS))

    natq0 = load_chunk(xq, 0, "q")
    natk0 = load_chunk(xkv, 0, "k")
    natq1 = load_chunk(xq, 1, "q2")
    natk1 = load_chunk(xkv, 1, "k2")
    transpose_nat(natq0, xqT, 0)
    transpose_nat(natk0, xkvT, 0)
    transpose_nat(natq1, xqT, 1)
    transpose_nat(natk1, xkvT, 1)
    # weight DMAs AFTER the chunk-1 transposes: the scalar hwdge queue
    # shares the ACT sequencer with the transpose copies, and dispatching
    # 18 weight DMAs first stalls the copies (and the first projection
    # matmul behind them) for ~11us
    _w, esink = load_rest_of_weights()
    wk1_sb, wk2_sb, wv1_sb, wv2_sb, wout_sb = (
        _w["k1"], _w["k2"], _w["v1"], _w["v2"], _w["out"])
    natq2 = load_chunk(xq, 2, "q")
    natk2 = load_chunk(xkv, 2, "k")
    natq3 = load_chunk(xq, 3, "q2")
    natk3 = load_chunk(xkv, 3, "k2")
    proj_groups(0)
    proj_v(0)
    transpose_nat(natq2, xqT, 2)
    transpose_nat(natk2, xkvT, 2)
    transpose_nat(natq3, xqT, 3)
    transpose_nat(natk3, xkvT, 3)
    proj_groups(1)
    proj_v(1)

    ps_tp.release()
    ps_w.release()
    wpool.release()
    xin.release()
    xt_p.release()

    # attend-phase pools (allocated after the phase-1 pools are released)
    e1p = tc.alloc_tile_pool(name="e1", bufs=1)    # 16 resident e tiles
    epool = tc.alloc_tile_pool(name="e", bufs=3)
    npool = tc.alloc_tile_pool(name="nrm", bufs=2)
    osb_p = tc.alloc_tile_pool(name="osb", bufs=2)
    ps_a = tc.alloc_tile_pool(name="ps_a", bufs=2, space="PSUM")   # 4 banks
    ps_b = tc.alloc_tile_pool(name="ps_b", bufs=1, space="PSUM")   # 2 banks
    ps_on = tc.alloc_tile_pool(name="ps_on", bufs=1, space="PSUM")  # 1 bank
    ps_bc = tc.alloc_tile_pool(name="ps_bc", bufs=1, space="PSUM")  # 1 bank
    _pools2 = [e1p, epool, npool, osb_p, ps_a, ps_b, ps_on, ps_bc]

    if PROJ_ONLY:
        for nb in range(NB):
            osb = osb_p.tile([128, DQ], FP32, tag="osb", name="osb")
            nc.vector.tensor_copy(osb[:, 0:512], v1_sb[nb][:])
            nc.vector.tensor_copy(osb[:, 512:1024], v1_sb[nb][:])
            nc.sync.dma_start(out=out[nb * 128:(nb + 1) * 128, :], in_=osb[:])
        for p_ in reversed(_pools2):
            p_.release()
        for p_ in (stat, const):
            p_.release()
        return

    # =====================================================================
    # Phase 2: attends (everything SBUF-resident)
    # =====================================================================
    def masked_exp_av(k_h, rhs_h, v_ap, out_ps, ones_ps, p):
        """One attend pass: for each key block jb, sim -> exp -> mask ->
        accumulate v.T @ e (and the ones row for attend1 denominators).

        Software-pipelined one jb deep: the PE emission order is
        sim(0), sim(1), av(0), sim(2), av(1), ... so the in-order PE queue
        never stalls on exp/mask of the block it is about to accumulate."""
        def do_sim(jb):
            simp = ps_a.tile([128, PASS], FP32, tag="sim", name="sim")
            for (a, b) in _mm_runs(jb, p):
                mm(simp[:, a:b],
                   k_h[:, jb * 128:(jb + 1) * 128],
                   rhs_h[:, a:b],
                   start=True, stop=True)
            return simp

        def do_e(jb, simp):
            runs, skip = _runs_for(jb, p)
            e = epool.tile([128, PASS], MM_DT, tag="e", name="e")
            for (t0, t1) in runs:
                nc.scalar.activation(e[:, t0 * 128:t1 * 128],
                                     simp[:, t0 * 128:t1 * 128], ACT.Exp)
            if skip is not None:
                nc.vector.memset(e[:, skip[0] * 128:skip[1] * 128], 0.0)
            td = jb - 8 * p
            if 0 <= td < 8:   # diagonal block: keep jj <= ii
                blk = e[:, td * 128:(td + 1) * 128]
                nc.vector.tensor_mul(blk, blk, tri_le[:])
            ta = jb - 4 - 8 * p
            if 0 <= ta < 8:   # jb == I+4 block: keep jj > ii
                blk = e[:, ta * 128:(ta + 1) * 128]
                nc.vector.tensor_mul(blk, blk, tri_gt[:])
            return e

        def do_av(jb, e):
            segs = ([(0, 512), (512, 1024)] if jb in (0, NB - 1)
                    else _mm_runs(jb, p))
            for (a, b) in segs:
                mm(out_ps[:, a:b],
                   v_ap(jb),
                   e[:, a:b],
                   start=(jb == 0), stop=(jb == NB - 1),
                   skip_group_check=True)
            if ones_ps is not None:
                for (a, b) in segs:
                    s = a // 512
                    mm(ones_ps[32 * s:32 * s + 1, a - 512 * s:b - 512 * s],
                       onescol[:], e[:, a:b],
                       start=(jb == 0), stop=(jb == NB - 1),
                       skip_group_check=True)

        prev = None
        for jb in range(NB):
            simp = do_sim(jb)
            if prev is not None:
                do_av(jb - 1, prev)
            prev = do_e(jb, simp)
        do_av(NB - 1, prev)

    def sim_exp_1(h, p):
        """Attend1 S-stage: sims -> exp -> mask into 16 resident e tiles.
        Emitted one pass ahead so the PE has independent work during the
        previous pass's normalization chain."""
        hh = 64 * (h % 2)
        k1h = k1T_sb[h // 2][hh:hh + 64, :]
        qh = qT_sb[h // 2][hh:hh + 64, p * PASS:(p + 1) * PASS]
        es = []
        for jb in range(NB):
            simp = ps_a.tile([128, PASS], FP32, tag="sim", name="sim")
            for (a, b) in _mm_runs(jb, p):
                mm(simp[:, a:b],
                   k1h[:, jb * 128:(jb + 1) * 128],
                   qh[:, a:b],
                   start=True, stop=True)
            e = e1p.tile([128, PASS], MM_DT, tag=f"e1_{jb}", name=f"e1_{jb}")
            runs, skip = _runs_for(jb, p)
            for (t0, t1) in runs:
                nc.scalar.activation(e[:, t0 * 128:t1 * 128],
                                     simp[:, t0 * 128:t1 * 128], ACT.Exp)
            if skip is not None:
                nc.vector.memset(e[:, skip[0] * 128:skip[1] * 128], 0.0)
            td = jb - 8 * p
            if 0 <= td < 8:
                blk = e[:, td * 128:(td + 1) * 128]
                nc.vector.tensor_mul(blk, blk, tri_le[:])
            ta = jb - 4 - 8 * p
            if 0 <= ta < 8:
                blk = e[:, ta * 128:(ta + 1) * 128]
                nc.vector.tensor_mul(blk, blk, tri_gt[:])
            es.append(e)
        return es

    def wout_half(p):
        """Phase 3 for the column half finished by pass group p."""
        for nb in range(8 * p, 8 * p + 8):
            pool, tag = (ps_b, "av") if nb % 2 == 0 else (ps_a, "sim")
            acc = pool.tile([128, PASS], FP32, tag=tag, name=tag)
            for s in range(2):
                for kt in range(2):
                    mm(acc[:, s * 512:(s + 1) * 512],
                       o2T[kt][:, nb * 128:(nb + 1) * 128],
                       wout_sb[kt][:, s * 512:(s + 1) * 512],
                       start=(kt == 0), stop=(kt == 1))
            osb = osb_p.tile([128, DQ], FP32, tag="osb", name="osb")
            if nb % 2 == 0:
                nc.vector.tensor_copy(osb[:], acc[:])
            else:
                nc.scalar.copy(osb[:], acc[:])
            nc.sync.dma_start(out=out[nb * 128:(nb + 1) * 128, :], in_=osb[:])

    passes = [(h, p) for p in range(2) for h in range(HEADS)]
    e1s = sim_exp_1(*passes[0])
    for idx, (h, p) in enumerate(passes):
        hh = 64 * (h % 2)

        # ------------- attend 1 V-stage: av + denominator matmuls ---------
        out1 = ps_b.tile([128, PASS], FP32, tag="av", name="av")
        ones = ps_on.tile([33, 512], FP32, tag="ones", name="ones")
        for jb in range(NB):
            segs = ([(0, 512), (512, 1024)] if jb in (0, NB - 1)
                    else _mm_runs(jb, p))
            for (a, b) in segs:
                mm(out1[:, a:b],
                   v1_sb[jb][:, 128 * h:128 * h + 128],
                   e1s[jb][:, a:b],
                   start=(jb == 0), stop=(jb == NB - 1),
                   skip_group_check=True)
            for (a, b) in segs:
                s = a // 512
                mm(ones[32 * s:32 * s + 1, a - 512 * s:b - 512 * s],
                   onescol[:], e1s[jb][:, a:b],
                   start=(jb == 0), stop=(jb == NB - 1),
                   skip_group_check=True)

        # normalize (z = out1 / denom) + silu -> hT, pipelined per
        # 512-column half: half 1's broadcast/copy overlaps half 0's DVE
        # chain, and attend2's first sim chunk can start on hT[:, 0:512]
        # while half 1 is still in flight.
        # silu(z) = z * sigmoid(z) = z / (1 + exp(-z)); stays in the
        # Exp activation table (Silu lives in a different table)
        zf = npool.tile([128, PASS], FP32, tag="z", name="z")
        rbs = npool.tile([128, PASS], FP32, tag="rb", name="rb")
        tql = npool.tile([128, PASS], FP32, tag="tq", name="tq")
        hT = npool.tile([128, PASS], MM_DT, tag="hT", name="hT")
        for s_ in range(2):
            sl = slice(s_ * 512, (s_ + 1) * 512)
            ds_ = npool.tile([1, PASS], FP32, tag="ds", name="ds")
            nc.vector.tensor_copy(ds_[0:1, 0:512], ones[32 * s_:32 * s_ + 1, :])
            nc.vector.tensor_scalar_add(ds_[0:1, 0:512], ds_[0:1, 0:512],
                                        esink[0:1, h:h + 1])
            nc.vector.reciprocal_approx_fast(ds_[0:1, 0:512], ds_[0:1, 0:512])
            rbp = ps_bc.tile([128, 512], FP32, tag="bc", name="bc")
            mm(rbp[:], onesrow[:], ds_[0:1, 0:512], start=True, stop=True)
            nc.scalar.copy(rbs[:, sl], rbp[:])
            nc.vector.tensor_mul(zf[:, sl], out1[:, sl], rbs[:, sl])
            nc.scalar.activation(tql[:, sl], zf[:, sl], ACT.Exp, scale=-1.0)
            nc.vector.tensor_scalar_add(tql[:, sl], tql[:, sl], 1.0)
            nc.vector.reciprocal_approx_fast(tql[:, sl], tql[:, sl])
            nc.vector.tensor_mul(hT[:, sl], zf[:, sl], tql[:, sl])

        # next pass's S-stage: fills the PE while the chain above runs
        if idx + 1 < len(passes):
            e1s = sim_exp_1(*passes[idx + 1])

        # ------------- attend 2 (fused jb-pipelined) -------------
        k2h = k2T_sb[h][:]
        out2 = ps_b.tile([65, PASS], FP32, tag="av", name="av")
        masked_exp_av(
            k2h, hT[:], lambda jb: v2a_sb[jb][:, 65 * h:65 * h + 65],
            out2[:], None, p)

        # normalize attend2 (denominator rode along as row 64)
        d2 = npool.tile([1, PASS], FP32, tag="ds", name="ds")
        nc.vector.tensor_copy(d2[:], out2[64:65, :])
        nc.vector.tensor_scalar_add(d2[:], d2[:], esink[0:1, h:h + 1])
        nc.vector.reciprocal_approx_fast(d2[:], d2[:])
        rbs2 = npool.tile([64, PASS], FP32, tag="rb2", name="rb2")
        for s_ in range(2):
            rbp = ps_bc.tile([128, 512], FP32, tag="bc", name="bc")
            mm(rbp[0:64, :], onesrow[0:1, 0:64],
               d2[0:1, s_ * 512:(s_ + 1) * 512], start=True, stop=True)
            nc.scalar.copy(rbs2[:, s_ * 512:(s_ + 1) * 512], rbp[0:64, :])
        dst = o2T[h // 2][hh:hh + 64, p * PASS:(p + 1) * PASS]
        nc.vector.tensor_mul(dst, out2[0:64, :], rbs2[:])

        if DEBUG and h == 0 and p == 0:
            nc.sync.dma_start(out=io["dbg_hT"].bitcast(MM_DT), in_=hT[:])
            dzf = npool.tile([128, PASS], FP32, tag="dzf", name="dzf")
            nc.vector.tensor_copy(dzf[:], zf[:])
            nc.sync.dma_start(out=io["dbg_zf"], in_=dzf[:])
            do2 = npool.tile([65, PASS], FP32, tag="do2", name="do2")
            nc.vector.tensor_copy(do2[:], out2[:])
            nc.sync.dma_start(out=io["dbg_out2"], in_=do2[:])

        # interleave the output projection for the completed column half
        if idx == len(passes) - 1 or (idx + 1 < len(passes)
                                      and passes[idx + 1][1] != p):
            wout_half(p)

    if DEBUG:
        for t in range(2):
            nc.sync.dma_start(out=io["dbg_qT"][t * 128:(t + 1) * 128, :].bitcast(MM_DT),
                              in_=qT_sb[t][:])
            nc.sync.dma_start(out=io["dbg_k1T"][t * 128:(t + 1) * 128, :].bitcast(MM_DT),
                              in_=k1T_sb[t][:])
            nc.sync.dma_start(out=io["dbg_o2T"][t * 128:(t + 1) * 128, :].bitcast(MM_DT),
                              in_=o2T[t][:])
        for t in range(4):
            nc.sync.dma_start(out=io["dbg_v1"][t * 128:(t + 1) * 128, :].bitcast(MM_DT),
                              in_=v1_sb[t][:])

    for p_ in reversed(_pools2):
        p_.release()
    for p_ in (stat, const):
        p_.release()


_NC_CACHE = {}


def build_nc():
    key = (str(MM_DT), REPS, DEBUG, PROJ_ONLY)
    if key in _NC_CACHE:
        return _NC_CACHE[key]
    nc = bacc.Bacc("TRN2", target_bir_lowering=False, debug=False,
                   num_devices=N_CORES)
    io = {
        "xq": nc.dram_tensor("xq", [N, DQ], MM_DT, kind="ExternalInput").ap(),
        "xkv": nc.dram_tensor("xkv", [N, DQ], MM_DT, kind="ExternalInput").ap(),
        "wq": nc.dram_tensor("wq", [DQ, 256], MM_DT, kind="ExternalInput").ap(),
        "wk1": nc.dram_tensor("wk1", [DQ, 256], MM_DT, kind="ExternalInput").ap(),
        "wv1": nc.dram_tensor("wv1", [DQ, 512], MM_DT, kind="ExternalInput").ap(),
        "wk2": nc.dram_tensor("wk2", [DQ, 512], MM_DT, kind="ExternalInput").ap(),
        "wv2": nc.dram_tensor("wv2", [DQ, 256], MM_DT, kind="ExternalInput").ap(),
        "wout": nc.dram_tensor("wout", [256, DQ], MM_DT, kind="ExternalInput").ap(),
        "sink": nc.dram_tensor("sink", [1, HEADS], FP32, kind="ExternalInput").ap(),
        "out": nc.dram_tensor("out", [N, DQ], FP32, kind="ExternalOutput").ap(),
    }
    if DEBUG:
        for nm, shp, dt in (("dbg_qT", [256, N], FP32), ("dbg_k1T", [256, N], FP32),
                            ("dbg_o2T", [256, N], FP32), ("dbg_v1", [512, 512], FP32),
                            ("dbg_hT", [128, PASS], FP32), ("dbg_zf", [128, PASS], FP32),
                            ("dbg_out2", [65, PASS], FP32)):
            shp2 = list(shp)
            if dt is FP32 and nm in ("dbg_qT", "dbg_k1T", "dbg_o2T", "dbg_v1", "dbg_hT"):
                shp2[-1] = shp[-1] // 2   # bf16 payload bitcast into fp32 words
            io[nm] = nc.dram_tensor(nm, shp2, FP32, kind="ExternalOutput").ap()
    if REPS == 0:
        # extra input so the I/O-only program's jax trace-cache key differs
        # from the real kernel's (the cache ignores the BIR payload)
        io["dummy0"] = nc.dram_tensor("dummy0", [1, 8], FP32,
                                      kind="ExternalInput").ap()
    with TileContext(nc) as tc:
        if REPS == 0:
            pool0 = tc.alloc_tile_pool(name="p0", bufs=1)
            t0_ = pool0.tile([128, DQ], MM_DT, name="t0_")
            nc.sync.dma_start(out=t0_[:], in_=io["xq"][0:128, :])
            o0_ = pool0.tile([128, DQ], FP32, name="o0_")
            nc.vector.tensor_copy(o0_[:], t0_[:])
            for nb in range(NB):
                nc.sync.dma_start(out=io["out"][nb * 128:(nb + 1) * 128, :],
                                  in_=o0_[:])
            pool0.release()
        for _ in range(REPS):
            build_kernel(nc, tc, io)
    nc.compile()
    _NC_CACHE[key] = (nc, io)
    return nc, io


_BF16 = None


def _bf16():
    global _BF16
    if _BF16 is None:
        import ml_dtypes
        _BF16 = np.dtype(ml_dtypes.bfloat16)
    return _BF16


def make_in_maps(inputs):
    bf = _bf16()
    xq_b = [np.ascontiguousarray(inputs["queries_input"][b]).astype(bf)
            for b in range(2)]
    xkv_b = [np.ascontiguousarray(inputs["key_values_input"][b]).astype(bf)
             for b in range(2)]
    in_maps = []
    for c in range(N_CORES):
        b, g = c // 4, c % 4
        s64 = slice(g * 256, (g + 1) * 256)
        s128 = slice(g * 512, (g + 1) * 512)
        in_maps.append({
            "xq": xq_b[b],
            "xkv": xkv_b[b],
            "wq": np.ascontiguousarray(inputs["Wq"][:, s64]).astype(bf),
            "wk1": np.ascontiguousarray(inputs["Wk1"][:, s64]).astype(bf),
            "wv1": np.ascontiguousarray(inputs["Wv1"][:, s128]).astype(bf),
            "wk2": np.ascontiguousarray(inputs["Wk2"][:, s128]).astype(bf),
            "wv2": np.ascontiguousarray(inputs["Wv2"][:, s64]).astype(bf),
            "wout": np.ascontiguousarray(inputs["Wout"][s64, :]).astype(bf),
            "sink": np.ascontiguousarray(
                inputs["attn_sink"][g * 4:(g + 1) * 4]).reshape(1, HEADS)
                .astype(np.float32),
        })
    return in_maps


def kernel(**inputs):
    from concourse.bass_utils import run_bass_kernel_spmd

    inputs = {k: np.asarray(v) for k, v in inputs.items()}
    nc, _ = build_nc()
    in_maps = make_in_maps(inputs)
    res = run_bass_kernel_spmd(nc, in_maps, list(range(N_CORES)))
    out = np.zeros((2, N, DQ), dtype=np.float32)
    for c in range(N_CORES):
        out[c // 4] += res.results[c]["out"]
    return out



# revision 6
# speedup vs baseline: 1.8407x; 1.2507x over previous
"""Trainium2 Bass kernel for nn_Attention_31997506355363 (sparse_attention).

Sharding: 8 cores = 2 batches x 4 head-groups (4 heads of 16 each).
Each core computes its batch's full-sequence double-attend for its 4 heads,
plus the partial output projection (Wout rows for its heads); host sums the
4 head-group partials per batch.

Math notes (verified vs reference):
  - mask keeps j<=i OR j>i+512  (the strip i<j<=i+512 is masked out)
  - softmax has a per-head sink logit in the denominator only
  - |sim| <= ~6.4 so softmax runs without max-subtraction: p = exp(sim),
    denom = sum_j p + exp(sink)
  - attends are computed transposed: simT[j,i] tiles -> exp -> outT
    accumulated as v.T @ p per 128-j-block (contraction always on the
    partition dim, so no attention-matrix transposes are needed, and
    attend1's output hiddensT feeds attend2 directly)

Perf structure (v2):
  - all matmul operands bf16 (fp32 PE runs at 1/4 rate; tolerance is 2e-2)
  - x transposed by XBAR DMA-transpose (2-byte dtype) straight into SBUF;
    no PE transposes, no PSUM->SBUF copies for xT
  - everything SBUF-resident between phases; weights loaded once;
    phase-1-only pools (xT, projection weights, wide PSUM accs) released
    before the attends
  - projections run stationary-major (one Ldweights per (w-slice), 4
    full-width moving matmuls) to cut PE sequencer pressure
  - masking via DVE multiplies with constant 0/1 triangular tiles + DVE
    memsets; GPSIMD only does one-time constant setup
  - softmax denominators: ones-row matmuls accumulate alongside v.T @ e;
    reciprocal broadcast back to 128 partitions via a rank-1 PE matmul
"""

import sys

for _p in ("/opt/trn_rl_repo",):
    if _p not in sys.path:
        sys.path.insert(0, _p)

import numpy as np
import concourse.bass as bass
from concourse import bacc
import concourse.mybir as mybir
from concourse.tile import TileContext
from concourse.masks import make_identity

FP32 = mybir.dt.float32
MM_DT = mybir.dt.bfloat16
N_CORES = 8
N = 2048            # sequence length
DQ = 1024           # model dim
HEADS = 4           # heads per core
SCALE = 0.125       # 64 ** -0.5, folded into k1T / k2T at projection copy
NB = N // 128       # 16 key blocks
PASS = 1024         # attend i-pass width (2 passes)
ACT = mybir.ActivationFunctionType

DEBUG = False
REPS = 1
PROJ_ONLY = False   # timing experiment: stop after projections


def _runs_for(jb, p):
    """i-subblock runs (in 128-col units within a 1024-wide pass) that are
    not fully masked for key-block jb.  Sub-block t covers queries
    I = 8p + t; (I, jb) is fully masked iff 1 <= jb - I <= 3."""
    skip_lo = max(0, jb - 8 * p - 3)
    skip_hi = min(8, jb - 8 * p)
    if skip_lo >= skip_hi:
        return [(0, 8)], None
    runs = []
    if skip_lo > 0:
        runs.append((0, skip_lo))
    if skip_hi < 8:
        runs.append((skip_hi, 8))
    return runs, (skip_lo, skip_hi)


def _mm_runs(jb, p):
    """Non-masked col ranges (elements, within the 1024-wide pass) for
    key-block jb, split at the 512 psum-bank boundary."""
    runs, _ = _runs_for(jb, p)
    out = []
    for (t0, t1) in runs:
        c0, c1 = t0 * 128, t1 * 128
        for h0, h1 in ((0, 512), (512, 1024)):
            a, b = max(c0, h0), min(c1, h1)
            if a < b:
                out.append((a, b))
    return out


def build_kernel(nc, tc, io):
    mm = nc.tensor.matmul

    xq, xkv = io["xq"], io["xkv"]
    wq, wk1, wv1, wk2, wv2, wout, sink = (
        io["wq"], io["wk1"], io["wv1"], io["wk2"], io["wv2"], io["wout"],
        io["sink"],
    )
    out = io["out"]

    const = tc.alloc_tile_pool(name="const", bufs=1)
    stat = tc.alloc_tile_pool(name="stat", bufs=1)
    # phase-1-only pools (released before the attends)
    xt_p = tc.alloc_tile_pool(name="xt", bufs=1)
    xin = tc.alloc_tile_pool(name="xin", bufs=1)
    wpool = tc.alloc_tile_pool(name="w", bufs=1)
    ps_w = tc.alloc_tile_pool(name="ps_w", bufs=2, space="PSUM")   # 4 banks
    ps_tp = tc.alloc_tile_pool(name="ps_tp", bufs=2, space="PSUM")  # 2 banks

    ident = const.tile([128, 128], MM_DT, tag="ident", name="ident")
    make_identity(nc, ident[:])

    # ---- constants ----
    onescol = const.tile([128, 1], MM_DT, tag="onescol", name="onescol")
    nc.vector.memset(onescol[:], 1.0)
    onesrow = const.tile([1, 128], FP32, tag="onesrow", name="onesrow")
    nc.vector.memset(onesrow[:], 1.0)
    ones4 = const.tile([128, HEADS], MM_DT, tag="ones4", name="ones4")
    nc.vector.memset(ones4[:], 1.0)

    # 0/1 triangular masks (e layout is [j partitions, i cols]):
    # tri_le keeps jj <= ii (diagonal block), tri_gt keeps jj > ii (block I+4)
    tri_le = const.tile([128, 128], MM_DT, tag="tri_le", name="tri_le")
    nc.gpsimd.memset(tri_le[:], 1.0)
    nc.gpsimd.affine_select(
        out=tri_le[:], in_=tri_le[:], compare_op=mybir.AluOpType.is_ge,
        fill=0.0, base=0, pattern=[[1, 128]], channel_multiplier=-1)
    tri_gt = const.tile([128, 128], MM_DT, tag="tri_gt", name="tri_gt")
    nc.gpsimd.memset(tri_gt[:], 1.0)
    nc.gpsimd.affine_select(
        out=tri_gt[:], in_=tri_gt[:], compare_op=mybir.AluOpType.is_ge,
        fill=0.0, base=-1, pattern=[[-1, 128]], channel_multiplier=1)

    # ---- weights (DMAs ordered around the transposes; see below) ----
    def load_w(w_dram, cols, nm, eng):
        wt = [wpool.tile([128, cols], MM_DT, tag=f"{nm}{kt}", name=f"{nm}{kt}")
              for kt in range(8)]
        for kt in range(8):
            e = eng if not isinstance(eng, tuple) else eng[kt % 2]
            e.dma_start(out=wt[kt][:], in_=w_dram[kt * 128:(kt + 1) * 128, :])
        return wt

    wq_sb = load_w(wq, 256, "wq", (nc.sync, nc.scalar))

    # ---- persistent SBUF intermediates ----
    qT_sb = [stat.tile([128, N], MM_DT, tag=f"qT{t}", name=f"qT{t}") for t in range(2)]
    k1T_sb = [stat.tile([128, N], MM_DT, tag=f"k1T{t}", name=f"k1T{t}") for t in range(2)]
    k2T_sb = [stat.tile([128, N], MM_DT, tag=f"k2T{t}", name=f"k2T{t}") for t in range(4)]
    v1_sb = [stat.tile([128, 512], MM_DT, tag=f"v1_{t}", name=f"v1_{t}") for t in range(NB)]
    v2a_sb = [stat.tile([128, 65 * HEADS], MM_DT, tag=f"v2a{t}", name=f"v2a{t}")
              for t in range(NB)]
    o2T = [stat.tile([128, N], MM_DT, tag=f"o2T{t}", name=f"o2T{t}") for t in range(2)]

    # =====================================================================
    # Phase 1: DMA-transpose x into SBUF, then stationary-major projections.
    # =====================================================================
    xqT = [xt_p.tile([128, N], MM_DT, tag=f"xqT{kt}", name=f"xqT{kt}")
           for kt in range(8)]
    xkvT = [xt_p.tile([128, N], MM_DT, tag=f"xkvT{kt}", name=f"xkvT{kt}")
            for kt in range(8)]

    def load_chunk(x_dram, c, qi):
        nat = []
        for nbl in range(4):
            r0 = c * 512 + nbl * 128
            t = xin.tile([128, DQ], MM_DT, tag=f"x{qi}{nbl}", name=f"x{qi}{nbl}")
            eng = nc.sync if (nbl % 2 == 0) else nc.scalar
            eng.dma_start(out=t[:], in_=x_dram[r0:r0 + 128, :])
            nat.append(t)
        return nat

    def transpose_nat(nat, xT, c):
        """PE-transpose a loaded 512-row chunk into xT[kt][:, c-cols].
        (The XBAR DMA-transpose path raced with compute consumers on HW —
        its completion semaphore does not reliably gate reads.)"""
        for kt in range(8):
            ps = ps_tp.tile([128, 512], MM_DT, tag="tp", name="tp")
            for nbl in range(4):
                nc.tensor.transpose(
                    ps[:, nbl * 128:(nbl + 1) * 128],
                    nat[nbl][:, kt * 128:(kt + 1) * 128], ident[:])
            if kt % 2 == 0:
                nc.vector.tensor_copy(xT[kt][:, c * 512:(c + 1) * 512], ps[:])
            else:
                nc.scalar.copy(xT[kt][:, c * 512:(c + 1) * 512], ps[:])

    def load_rest_of_weights():
        # emitted after the first chunk's x loads so the PE isn't starved
        # at startup waiting for transposable data behind 40 weight DMAs
        w = {}
        w["k1"] = load_w(wk1, 256, "wk1", nc.sync)
        w["k2"] = load_w(wk2, 512, "wk2", nc.scalar)
        w["v1"] = load_w(wv1, 512, "wv1", nc.sync)
        w["v2"] = load_w(wv2, 256, "wv2", nc.scalar)
        w["out"] = [stat.tile([128, DQ], MM_DT, tag=f"wo{t}", name=f"wo{t}")
                    for t in range(2)]
        for t in range(2):
            nc.scalar.dma_start(out=w["out"][t][:],
                                in_=wout[t * 128:(t + 1) * 128, :])
        sink_sb = const.tile([1, HEADS], FP32, tag="sink", name="sink")
        nc.scalar.dma_start(out=sink_sb[:], in_=sink[:])
        esink = const.tile([1, HEADS], FP32, tag="esink", name="esink")
        nc.scalar.activation(esink[:], sink_sb[:], ACT.Exp)
        return w, esink

    # q/k1/k2 groups: stationary-major (one Ldweights per (w-slice, kt, half),
    # two 512-wide moving matmuls); v1+v2 fused on a shared stationary.
    def proj_groups(hf):
        groups = (
            [(qT_sb[m], wq_sb, m, xqT, None) for m in range(2)]
            + [(k1T_sb[m], wk1_sb, m, xkvT, SCALE) for m in range(2)]
            + [(k2T_sb[m], wk2_sb, m, xkvT, SCALE) for m in range(4)]
        )
        cols = slice(hf * 1024, (hf + 1) * 1024)
        for gi, (dst, wsb, m, xT, scale) in enumerate(groups):
            acc = ps_w.tile([128, PASS], FP32, tag="pw", name="pw")
            for kt in range(8):
                for cb in range(2):
                    c0 = hf * 1024 + cb * 512
                    mm(acc[:, cb * 512:(cb + 1) * 512],
                       wsb[kt][:, m * 128:(m + 1) * 128],
                       xT[kt][:, c0:c0 + 512],
                       start=(kt == 0), stop=(kt == 7))
            if scale is None:
                if gi % 2 == 0:
                    nc.vector.tensor_copy(dst[:, cols], acc[:])
                else:
                    nc.scalar.copy(dst[:, cols], acc[:])
            else:
                if gi % 2 == 0:
                    nc.vector.tensor_scalar_mul(dst[:, cols], acc[:], scale)
                else:
                    nc.scalar.mul(dst[:, cols], acc[:], scale)

    def proj_v(hf):
        for nb in range(8 * hf, 8 * hf + 8):
            acc = ps_w.tile([128, PASS], FP32, tag="pw", name="pw")
            for kt in range(8):
                mm(acc[:, 0:512], xkvT[kt][:, nb * 128:(nb + 1) * 128], wv1_sb[kt][:],
                   start=(kt == 0), stop=(kt == 7))
                mm(acc[:, 512:768], xkvT[kt][:, nb * 128:(nb + 1) * 128], wv2_sb[kt][:],
                   start=(kt == 0), stop=(kt == 7))
            if nb % 2 == 0:
                nc.vector.tensor_copy(v1_sb[nb][:], acc[:, 0:512])
            else:
                nc.scalar.copy(v1_sb[nb][:], acc[:, 0:512])
            # pack v2 [h*64 cols] into 65-col groups with a ones column
            sv = v2a_sb[nb][:].rearrange("p (h c) -> p h c", h=HEADS)
            nc.vector.tensor_copy(
                sv[:, :, 0:64],
                acc[:, 512:768].rearrange("p (h c) -> p h c", h=HEADS))
            nc.vector.tensor_copy(
                sv[:, :, 64:65],
                ones4[:].rearrange("p (h c) -> p h c", h=HEADS))

    natq0 = load_chunk(xq, 0, "q")
    natk0 = load_chunk(xkv, 0, "k")
    natq1 = load_chunk(xq, 1, "q2")
    natk1 = load_chunk(xkv, 1, "k2")
    transpose_nat(natq0, xqT, 0)
    transpose_nat(natk0, xkvT, 0)
    transpose_nat(natq1, xqT, 1)
    transpose_nat(natk1, xkvT, 1)
    # weight DMAs AFTER the chunk-1 transposes: the scalar hwdge queue
    # shares the ACT sequencer with the transpose copies, and dispatching
    # 18 weight DMAs first stalls the copies (and the first projection
    # matmul behind them) for ~11us
    _w, esink = load_rest_of_weights()
    wk1_sb, wk2_sb, wv1_sb, wv2_sb, wout_sb = (
        _w["k1"], _w["k2"], _w["v1"], _w["v2"], _w["out"])
    natq2 = load_chunk(xq, 2, "q")
    natk2 = load_chunk(xkv, 2, "k")
    natq3 = load_chunk(xq, 3, "q2")
    natk3 = load_chunk(xkv, 3, "k2")
    proj_groups(0)
    proj_v(0)
    transpose_nat(natq2, xqT, 2)
    transpose_nat(natk2, xkvT, 2)
    transpose_nat(natq3, xqT, 3)
    transpose_nat(natk3, xkvT, 3)
    proj_groups(1)
    proj_v(1)

    ps_tp.release()
    ps_w.release()
    wpool.release()
    xin.release()
    xt_p.release()

    # attend-phase pools (allocated after the phase-1 pools are released)
    e1p = tc.alloc_tile_pool(name="e1", bufs=1)    # 16 resident e tiles
    epool = tc.alloc_tile_pool(name="e", bufs=3)
    npool = tc.alloc_tile_pool(name="nrm", bufs=2)
    osb_p = tc.alloc_tile_pool(name="osb", bufs=2)
    ps_a = tc.alloc_tile_pool(name="ps_a", bufs=2, space="PSUM")   # 4 banks
    ps_b = tc.alloc_tile_pool(name="ps_b", bufs=1, space="PSUM")   # 2 banks
    ps_on = tc.alloc_tile_pool(name="ps_on", bufs=1, space="PSUM")  # 1 bank
    ps_bc = tc.alloc_tile_pool(name="ps_bc", bufs=1, space="PSUM")  # 1 bank
    _pools2 = [e1p, epool, npool, osb_p, ps_a, ps_b, ps_on, ps_bc]

    if PROJ_ONLY:
        for nb in range(NB):
            osb = osb_p.tile([128, DQ], FP32, tag="osb", name="osb")
            nc.vector.tensor_copy(osb[:, 0:512], v1_sb[nb][:])
            nc.vector.tensor_copy(osb[:, 512:1024], v1_sb[nb][:])
            nc.sync.dma_start(out=out[nb * 128:(nb + 1) * 128, :], in_=osb[:])
        for p_ in reversed(_pools2):
            p_.release()
        for p_ in (stat, const):
            p_.release()
        return

    # =====================================================================
    # Phase 2: attends (everything SBUF-resident)
    # =====================================================================
    def masked_exp_av(k_h, rhs_h, v_ap, out_ps, ones_ps, p):
        """One attend pass: for each key block jb, sim -> exp -> mask ->
        accumulate v.T @ e (and the ones row for attend1 denominators).

        Software-pipelined one jb deep: the PE emission order is
        sim(0), sim(1), av(0), sim(2), av(1), ... so the in-order PE queue
        never stalls on exp/mask of the block it is about to accumulate."""
        def do_sim(jb):
            simp = ps_a.tile([128, PASS], FP32, tag="sim", name="sim")
            for (a, b) in _mm_runs(jb, p):
                mm(simp[:, a:b],
                   k_h[:, jb * 128:(jb + 1) * 128],
                   rhs_h[:, a:b],
                   start=True, stop=True)
            return simp

        def do_e(jb, simp):
            runs, skip = _runs_for(jb, p)
            e = epool.tile([128, PASS], MM_DT, tag="e", name="e")
            for (t0, t1) in runs:
                nc.scalar.activation(e[:, t0 * 128:t1 * 128],
                                     simp[:, t0 * 128:t1 * 128], ACT.Exp)
            if skip is not None:
                nc.vector.memset(e[:, skip[0] * 128:skip[1] * 128], 0.0)
            td = jb - 8 * p
            if 0 <= td < 8:   # diagonal block: keep jj <= ii
                blk = e[:, td * 128:(td + 1) * 128]
                nc.vector.tensor_mul(blk, blk, tri_le[:])
            ta = jb - 4 - 8 * p
            if 0 <= ta < 8:   # jb == I+4 block: keep jj > ii
                blk = e[:, ta * 128:(ta + 1) * 128]
                nc.vector.tensor_mul(blk, blk, tri_gt[:])
            return e

        def do_av(jb, e):
            segs = ([(0, 512), (512, 1024)] if jb in (0, NB - 1)
                    else _mm_runs(jb, p))
            for (a, b) in segs:
                mm(out_ps[:, a:b],
                   v_ap(jb),
                   e[:, a:b],
                   start=(jb == 0), stop=(jb == NB - 1),
                   skip_group_check=True)
            if ones_ps is not None:
                for (a, b) in segs:
                    s = a // 512
                    mm(ones_ps[32 * s:32 * s + 1, a - 512 * s:b - 512 * s],
                       onescol[:], e[:, a:b],
                       start=(jb == 0), stop=(jb == NB - 1),
                       skip_group_check=True)

        prev = None
        for jb in range(NB):
            simp = do_sim(jb)
            if prev is not None:
                do_av(jb - 1, prev)
            prev = do_e(jb, simp)
        do_av(NB - 1, prev)

    def sim_exp_1(h, p):
        """Attend1 S-stage: sims -> exp -> mask into 16 resident e tiles.
        Emitted one pass ahead so the PE has independent work during the
        previous pass's normalization chain."""
        hh = 64 * (h % 2)
        k1h = k1T_sb[h // 2][hh:hh + 64, :]
        qh = qT_sb[h // 2][hh:hh + 64, p * PASS:(p + 1) * PASS]
        es = []
        for jb in range(NB):
            simp = ps_a.tile([128, PASS], FP32, tag="sim", name="sim")
            for (a, b) in _mm_runs(jb, p):
                mm(simp[:, a:b],
                   k1h[:, jb * 128:(jb + 1) * 128],
                   qh[:, a:b],
                   start=True, stop=True)
            e = e1p.tile([128, PASS], MM_DT, tag=f"e1_{jb}", name=f"e1_{jb}")
            runs, skip = _runs_for(jb, p)
            for (t0, t1) in runs:
                nc.scalar.activation(e[:, t0 * 128:t1 * 128],
                                     simp[:, t0 * 128:t1 * 128], ACT.Exp)
            if skip is not None:
                nc.vector.memset(e[:, skip[0] * 128:skip[1] * 128], 0.0)
            td = jb - 8 * p
            if 0 <= td < 8:
                blk = e[:, td * 128:(td + 1) * 128]
                nc.vector.tensor_mul(blk, blk, tri_le[:])
            ta = jb - 4 - 8 * p
            if 0 <= ta < 8:
                blk = e[:, ta * 128:(ta + 1) * 128]
                nc.vector.tensor_mul(blk, blk, tri_gt[:])
            es.append(e)
        return es

    def wout_half(p):
        """Phase 3 for the column half finished by pass group p."""
        for nb in range(8 * p, 8 * p + 8):
            pool, tag = (ps_b, "av") if nb % 2 == 0 else (ps_a, "sim")
            acc = pool.tile([128, PASS], FP32, tag=tag, name=tag)
            for s in range(2):
                for kt in range(2):
                    mm(acc[:, s * 512:(s + 1) * 512],
                       o2T[kt][:, nb * 128:(nb + 1) * 128],
                       wout_sb[kt][:, s * 512:(s + 1) * 512],
                       start=(kt == 0), stop=(kt == 1))
            osb = osb_p.tile([128, DQ], FP32, tag="osb", name="osb")
            if nb % 2 == 0:
                nc.vector.tensor_copy(osb[:], acc[:])
            else:
                nc.scalar.copy(osb[:], acc[:])
            nc.sync.dma_start(out=out[nb * 128:(nb + 1) * 128, :], in_=osb[:])

    passes = [(h, p) for p in range(2) for h in range(HEADS)]
    e1s = sim_exp_1(*passes[0])
    for idx, (h, p) in enumerate(passes):
        hh = 64 * (h % 2)

        # ------------- attend 1 V-stage: av + denominator matmuls ---------
        out1 = ps_b.tile([128, PASS], FP32, tag="av", name="av")
        ones = ps_on.tile([33, 512], FP32, tag="ones", name="ones")
        # all av matmuls first, then all denominator matmuls: onescol's
        # stationary is loaded once instead of alternating with v1 every jb
        for jb in range(NB):
            segs = ([(0, 512), (512, 1024)] if jb in (0, NB - 1)
                    else _mm_runs(jb, p))
            for (a, b) in segs:
                mm(out1[:, a:b],
                   v1_sb[jb][:, 128 * h:128 * h + 128],
                   e1s[jb][:, a:b],
                   start=(jb == 0), stop=(jb == NB - 1),
                   skip_group_check=True)
        for jb in range(NB):
            segs = ([(0, 512), (512, 1024)] if jb in (0, NB - 1)
                    else _mm_runs(jb, p))
            for (a, b) in segs:
                s = a // 512
                mm(ones[32 * s:32 * s + 1, a - 512 * s:b - 512 * s],
                   onescol[:], e1s[jb][:, a:b],
                   start=(jb == 0), stop=(jb == NB - 1),
                   skip_group_check=True)

        # normalize (z = out1 / denom) + silu -> hT, pipelined per
        # 512-column half: half 1's broadcast/copy overlaps half 0's DVE
        # chain, and attend2's first sim chunk can start on hT[:, 0:512]
        # while half 1 is still in flight.
        # silu(z) = z * sigmoid(z) = z / (1 + exp(-z)); stays in the
        # Exp activation table (Silu lives in a different table)
        zf = npool.tile([128, PASS], FP32, tag="z", name="z")
        rbs = npool.tile([128, PASS], FP32, tag="rb", name="rb")
        tql = npool.tile([128, PASS], FP32, tag="tq", name="tq")
        hT = npool.tile([128, PASS], MM_DT, tag="hT", name="hT")
        for s_ in range(2):
            sl = slice(s_ * 512, (s_ + 1) * 512)
            ds_ = npool.tile([1, PASS], FP32, tag="ds", name="ds")
            nc.vector.tensor_copy(ds_[0:1, 0:512], ones[32 * s_:32 * s_ + 1, :])
            nc.vector.tensor_scalar_add(ds_[0:1, 0:512], ds_[0:1, 0:512],
                                        esink[0:1, h:h + 1])
            nc.vector.reciprocal_approx_fast(ds_[0:1, 0:512], ds_[0:1, 0:512])
            rbp = ps_bc.tile([128, 512], FP32, tag="bc", name="bc")
            mm(rbp[:], onesrow[:], ds_[0:1, 0:512], start=True, stop=True)
            nc.scalar.copy(rbs[:, sl], rbp[:])
            nc.vector.tensor_mul(zf[:, sl], out1[:, sl], rbs[:, sl])
            nc.scalar.activation(tql[:, sl], zf[:, sl], ACT.Exp, scale=-1.0)
            nc.vector.tensor_scalar_add(tql[:, sl], tql[:, sl], 1.0)
            nc.vector.reciprocal_approx_fast(tql[:, sl], tql[:, sl])
            nc.vector.tensor_mul(hT[:, sl], zf[:, sl], tql[:, sl])

        # next pass's S-stage: fills the PE while the chain above runs
        if idx + 1 < len(passes):
            e1s = sim_exp_1(*passes[idx + 1])

        # ------------- attend 2 (fused jb-pipelined) -------------
        k2h = k2T_sb[h][:]
        out2 = ps_b.tile([65, PASS], FP32, tag="av", name="av")
        masked_exp_av(
            k2h, hT[:], lambda jb: v2a_sb[jb][:, 65 * h:65 * h + 65],
            out2[:], None, p)

        # normalize attend2 (denominator rode along as row 64)
        d2 = npool.tile([1, PASS], FP32, tag="ds", name="ds")
        nc.vector.tensor_copy(d2[:], out2[64:65, :])
        nc.vector.tensor_scalar_add(d2[:], d2[:], esink[0:1, h:h + 1])
        nc.vector.reciprocal_approx_fast(d2[:], d2[:])
        rbs2 = npool.tile([64, PASS], FP32, tag="rb2", name="rb2")
        for s_ in range(2):
            rbp = ps_bc.tile([128, 512], FP32, tag="bc", name="bc")
            mm(rbp[0:64, :], onesrow[0:1, 0:64],
               d2[0:1, s_ * 512:(s_ + 1) * 512], start=True, stop=True)
            nc.scalar.copy(rbs2[:, s_ * 512:(s_ + 1) * 512], rbp[0:64, :])
        dst = o2T[h // 2][hh:hh + 64, p * PASS:(p + 1) * PASS]
        nc.vector.tensor_mul(dst, out2[0:64, :], rbs2[:])

        if DEBUG and h == 0 and p == 0:
            nc.sync.dma_start(out=io["dbg_hT"].bitcast(MM_DT), in_=hT[:])
            dzf = npool.tile([128, PASS], FP32, tag="dzf", name="dzf")
            nc.vector.tensor_copy(dzf[:], zf[:])
            nc.sync.dma_start(out=io["dbg_zf"], in_=dzf[:])
            do2 = npool.tile([65, PASS], FP32, tag="do2", name="do2")
            nc.vector.tensor_copy(do2[:], out2[:])
            nc.sync.dma_start(out=io["dbg_out2"], in_=do2[:])

        # interleave the output projection for the completed column half
        if idx == len(passes) - 1 or (idx + 1 < len(passes)
                                      and passes[idx + 1][1] != p):
            wout_half(p)

    if DEBUG:
        for t in range(2):
            nc.sync.dma_start(out=io["dbg_qT"][t * 128:(t + 1) * 128, :].bitcast(MM_DT),
                              in_=qT_sb[t][:])
            nc.sync.dma_start(out=io["dbg_k1T"][t * 128:(t + 1) * 128, :].bitcast(MM_DT),
                              in_=k1T_sb[t][:])
            nc.sync.dma_start(out=io["dbg_o2T"][t * 128:(t + 1) * 128, :].bitcast(MM_DT),
                              in_=o2T[t][:])
        for t in range(4):
            nc.sync.dma_start(out=io["dbg_v1"][t * 128:(t + 1) * 128, :].bitcast(MM_DT),
                              in_=v1_sb[t][:])

    for p_ in reversed(_pools2):
        p_.release()
    for p_ in (stat, const):
        p_.release()


_NC_CACHE = {}


def build_nc():
    key = (str(MM_DT), REPS, DEBUG, PROJ_ONLY)
    if key in _NC_CACHE:
        return _NC_CACHE[key]
    nc = bacc.Bacc("TRN2", target_bir_lowering=False, debug=False,
                   num_devices=N_CORES)
    io = {
        "xq": nc.dram_tensor("xq", [N, DQ], MM_DT, kind="ExternalInput").ap(),
        "xkv": nc.dram_tensor("xkv", [N, DQ], MM_DT, kind="ExternalInput").ap(),
        "wq": nc.dram_tensor("wq", [DQ, 256], MM_DT, kind="ExternalInput").ap(),
        "wk1": nc.dram_tensor("wk1", [DQ, 256], MM_DT, kind="ExternalInput").ap(),
        "wv1": nc.dram_tensor("wv1", [DQ, 512], MM_DT, kind="ExternalInput").ap(),
        "wk2": nc.dram_tensor("wk2", [DQ, 512], MM_DT, kind="ExternalInput").ap(),
        "wv2": nc.dram_tensor("wv2", [DQ, 256], MM_DT, kind="ExternalInput").ap(),
        "wout": nc.dram_tensor("wout", [256, DQ], MM_DT, kind="ExternalInput").ap(),
        "sink": nc.dram_tensor("sink", [1, HEADS], FP32, kind="ExternalInput").ap(),
        "out": nc.dram_tensor("out", [N, DQ], FP32, kind="ExternalOutput").ap(),
    }
    if DEBUG:
        for nm, shp, dt in (("dbg_qT", [256, N], FP32), ("dbg_k1T", [256, N], FP32),
                            ("dbg_o2T", [256, N], FP32), ("dbg_v1", [512, 512], FP32),
                            ("dbg_hT", [128, PASS], FP32), ("dbg_zf", [128, PASS], FP32),
                            ("dbg_out2", [65, PASS], FP32)):
            shp2 = list(shp)
            if dt is FP32 and nm in ("dbg_qT", "dbg_k1T", "dbg_o2T", "dbg_v1", "dbg_hT"):
                shp2[-1] = shp[-1] // 2   # bf16 payload bitcast into fp32 words
            io[nm] = nc.dram_tensor(nm, shp2, FP32, kind="ExternalOutput").ap()
    if REPS == 0:
        # extra input so the I/O-only program's jax trace-cache key differs
        # from the real kernel's (the cache ignores the BIR payload)
        io["dummy0"] = nc.dram_tensor("dummy0", [1, 8], FP32,
                                      kind="ExternalInput").ap()
    with TileContext(nc) as tc:
        if REPS == 0:
            pool0 = tc.alloc_tile_pool(name="p0", bufs=1)
            t0_ = pool0.tile([128, DQ], MM_DT, name="t0_")
            nc.sync.dma_start(out=t0_[:], in_=io["xq"][0:128, :])
            o0_ = pool0.tile([128, DQ], FP32, name="o0_")
            nc.vector.tensor_copy(o0_[:], t0_[:])
            for nb in range(NB):
                nc.sync.dma_start(out=io["out"][nb * 128:(nb + 1) * 128, :],
                                  in_=o0_[:])
            pool0.release()
        for _ in range(REPS):
            build_kernel(nc, tc, io)
    nc.compile()
    _NC_CACHE[key] = (nc, io)
    return nc, io


_BF16 = None


def _bf16():
    global _BF16
    if _BF16 is None:
        import ml_dtypes
        _BF16 = np.dtype(ml_dtypes.bfloat16)
    return _BF16


def make_in_maps(inputs):
    bf = _bf16()
    xq_b = [np.ascontiguousarray(inputs["queries_input"][b]).astype(bf)
            for b in range(2)]
    xkv_b = [np.ascontiguousarray(inputs["key_values_input"][b]).astype(bf)
             for b in range(2)]
    in_maps = []
    for c in range(N_CORES):
        b, g = c // 4, c % 4
        s64 = slice(g * 256, (g + 1) * 256)
        s128 = slice(g * 512, (g + 1) * 512)
        in_maps.append({
            "xq": xq_b[b],
            "xkv": xkv_b[b],
            "wq": np.ascontiguousarray(inputs["Wq"][:, s64]).astype(bf),
            "wk1": np.ascontiguousarray(inputs["Wk1"][:, s64]).astype(bf),
            "wv1": np.ascontiguousarray(inputs["Wv1"][:, s128]).astype(bf),
            "wk2": np.ascontiguousarray(inputs["Wk2"][:, s128]).astype(bf),
            "wv2": np.ascontiguousarray(inputs["Wv2"][:, s64]).astype(bf),
            "wout": np.ascontiguousarray(inputs["Wout"][s64, :]).astype(bf),
            "sink": np.ascontiguousarray(
                inputs["attn_sink"][g * 4:(g + 1) * 4]).reshape(1, HEADS)
                .astype(np.float32),
        })
    return in_maps


def kernel(**inputs):
    from concourse.bass_utils import run_bass_kernel_spmd

    inputs = {k: np.asarray(v) for k, v in inputs.items()}
    nc, _ = build_nc()
    in_maps = make_in_maps(inputs)
    res = run_bass_kernel_spmd(nc, in_maps, list(range(N_CORES)))
    out = np.zeros((2, N, DQ), dtype=np.float32)
    for c in range(N_CORES):
        out[c // 4] += res.results[c]["out"]
    return out

